# revision 13
# baseline (speedup 1.0000x reference)
"""Multi-head attention forward on 8 Trainium2 NeuronCores (Bass/Tile).

Problem: x[4, 2048, 768] -> qkv proj (w_qkv[2304, 768]) -> 12-head attention
(softmax((q k^T) * 768^-0.5)) -> out proj (w_out[768, 768]).

Sharding: core c handles batch b = c//2 and a group of 6 heads g = c%2
(tensor parallel over heads within a batch pair). Each core computes a
partial output (its heads' contribution through the row-sliced out
projection, transposed: [768, 2048]); the host sums the two partials per
batch, transposes back and adds b_out.

Device-side layout notes (everything transposed so the contraction dim sits
on SBUF partitions):
  xT   [768, 2048]  built on-chip via PE transposes of x tiles
  qkvT [feat, 2048] = wT.T @ xT via fp32r matmuls (full-rate fp32)
  scoresT[keys, q]  = kT_tile.T @ qT  (so attn@v needs no transpose)
  softmax without max-subtraction (scores are O(1); exp is safe in fp32);
  denominator comes free from an appended ones-column in v ("v_aug"),
  divide folded into the PSUM->SBUF copyback on DVE.
"""

import os
import sys

import numpy as np

if "/opt/trn_rl_repo" not in sys.path:
    sys.path.insert(0, "/opt/trn_rl_repo")

B = 4
N = 2048
DIM = 768
HEADS = 12
DHEAD = 64
SCALE = DIM ** (-0.5)
NCORES = 8
HPC = 6  # heads per core
FEAT = HPC * DHEAD  # 384 per-core attention features

_PROGRAM = None  # (nc,) cached compiled bass program


def _build_program():
    from contextlib import ExitStack

    import concourse.bass as bass
    import concourse.tile as tile
    from concourse import bacc, mybir
    from concourse.masks import make_identity

    f32 = mybir.dt.float32
    f32r = mybir.dt.float32r
    Alu = mybir.AluOpType
    ActF = mybir.ActivationFunctionType

    def r(ap):
        return ap

    nc = bacc.Bacc("TRN2", target_bir_lowering=False, debug=False)

    x_in = nc.dram_tensor("x", [N, DIM], f32, kind="ExternalInput")
    wqkvT = nc.dram_tensor("wqkvT", [DIM, 3 * FEAT], f32r, kind="ExternalInput")
    bqkv = nc.dram_tensor("bqkv", [128, 9], f32, kind="ExternalInput")
    woutT = nc.dram_tensor("woutT", [FEAT, DIM], f32r, kind="ExternalInput")
    out_T = nc.dram_tensor("outT", [DIM, N], f32, kind="ExternalOutput")

    NT = N // 128  # 16 n-tiles
    KC = DIM // 128  # 6 contraction chunks for dim
    NSPAN = N // 512  # 4 moving spans

    with tile.TileContext(nc) as tc, ExitStack() as ctx:
        const = ctx.enter_context(tc.tile_pool(name="const", bufs=1))
        identity = const.tile([128, 128], f32)
        make_identity(nc, identity)
        ones_f32 = const.tile([128, 1], f32)
        nc.vector.memset(ones_f32[:, :], 1.0)
        ones65 = const.tile([65, 64], f32r)
        nc.vector.tensor_copy(
            out=ones65[:, :], in_=ones_f32[0:65, :].to_broadcast((65, 64))
        )
        bias_sb = const.tile([128, 9], f32)
        nc.sync.dma_start(bias_sb[:, :], bqkv[:, :])

        wpool = ctx.enter_context(tc.tile_pool(name="w", bufs=1))
        w_all = wpool.tile([128, KC, 3 * FEAT], f32r)
        for j in range(KC):
            nc.sync.dma_start(w_all[:, j, :], wqkvT[j * 128 : (j + 1) * 128, :])
        wout_sb = wpool.tile([128, 3, DIM], f32r)
        for c in range(3):
            nc.sync.dma_start(wout_sb[:, c, :], woutT[c * 128 : (c + 1) * 128, :])

        # PSUM pools: spool 2x[128,2,512] (4 banks) + opool 3x[65/64,512] (3)
        spool = ctx.enter_context(tc.tile_pool(name="spsum", bufs=2, space="PSUM"))
        opool = ctx.enter_context(tc.tile_pool(name="opsum", bufs=3, space="PSUM"))

        xt_pool = ctx.enter_context(tc.tile_pool(name="xT", bufs=1))
        xT = xt_pool.tile([128, KC, N], f32r)  # xT[p, j, n] = x[n, j*128+p]

        xin_pool = ctx.enter_context(tc.tile_pool(name="xin", bufs=2))

        # Phase 1: load x and transpose on PE into xT
        for i in range(NT):
            xin = xin_pool.tile([128, DIM], f32)
            nc.sync.dma_start(xin[:, :], x_in[i * 128 : (i + 1) * 128, :])
            for j in range(KC):
                tp = spool.tile([128, 2, 512], f32, tag="s")
                nc.tensor.transpose(
                    tp[:, 0, 0:128], xin[:, j * 128 : (j + 1) * 128], identity[:, :]
                )
                nc.vector.tensor_copy(
                    out=xT[:, j, i * 128 : (i + 1) * 128], in_=tp[:, 0, 0:128]
                )

        qk_pool = ctx.enter_context(tc.tile_pool(name="qk", bufs=1))
        vt_pool = ctx.enter_context(tc.tile_pool(name="vt", bufs=1))
        vaug_pool = ctx.enter_context(tc.tile_pool(name="vaug", bufs=2))
        exp_pool = ctx.enter_context(tc.tile_pool(name="expT", bufs=4))
        rcp_pool = ctx.enter_context(tc.tile_pool(name="rcp", bufs=2))
        sbo_pool = ctx.enter_context(tc.tile_pool(name="sbo", bufs=2))
        hst_pool = ctx.enter_context(tc.tile_pool(name="hstage", bufs=1))
        ao_pool = ctx.enter_context(tc.tile_pool(name="attnout", bufs=1))
        attn_outT = ao_pool.tile([128, 3, N], f32r)

        for hp in range(3):  # head pairs
            # ---- qkv projection for this head pair ----
            # M-tiles: q feat tile hp, k tile 3+hp, v tile 6+hp
            qk = qk_pool.tile([128, 2, N], f32r)  # [:,0,:]=qT pair, [:,1,:]=kT pair
            vT = vt_pool.tile([128, N], f32)
            for idx, m in ((0, hp), (1, 3 + hp), (2, 6 + hp)):
                for sp2 in range(NSPAN // 2):  # pairs of 512-spans
                    ps = spool.tile([128, 2, 512], f32, tag="s")
                    for u in range(2):
                        span = 2 * sp2 + u
                        for j in range(KC):
                            nc.tensor.matmul(
                                ps[:, u, :],
                                r(w_all[:, j, m * 128 : (m + 1) * 128]),
                                r(xT[:, j, span * 512 : (span + 1) * 512]),
                                start=(j == 0),
                                stop=(j == KC - 1),
                            )
                    if idx < 2:
                        dst = qk[:, idx, sp2 * 1024 : (sp2 + 1) * 1024]
                    else:
                        dst = vT[:, sp2 * 1024 : (sp2 + 1) * 1024]
                    nc.vector.tensor_scalar(
                        dst.rearrange("p (a b) -> p a b", a=2),
                        ps[:, :, :],
                        bias_sb[:, m : m + 1],
                        None,
                        Alu.add,
                    )

            # ---- v transpose into v_aug [keys, 2*65] with ones columns ----
            vaug = vaug_pool.tile([128, NT, 130], f32r)
            ones_cols = vaug[:, :, :].rearrange("p k (t c) -> p k t c", t=2)[
                :, :, :, 64:65
            ]
            nc.vector.tensor_copy(
                out=ones_cols, in_=ones_f32[:, :].to_broadcast((128, NT, 2, 1))
            )
            for kc in range(NT):
                tp = spool.tile([128, 2, 512], f32, tag="s")
                nc.tensor.transpose(
                    tp[:, 0, 0:128], vT[:, kc * 128 : (kc + 1) * 128], identity[:, :]
                )
                nc.vector.tensor_copy(
                    out=vaug[:, kc, :].rearrange("p (t c) -> p t c", t=2)[:, :, 0:64],
                    in_=tp[:, 0, 0:128].rearrange("p (t c) -> p t c", t=2),
                )

            # ---- attention for the two heads of this pair ----
            for j in range(2):
                lo, hi = j * 64, (j + 1) * 64
                qT = qk[lo:hi, 0, :]
                kT = qk[lo:hi, 1, :]
                # DVE cannot write partitions 64:128 while reading 0:64, so
                # odd heads go through a base-0 staging tile + SBUF->SBUF DMA.
                hstage = (
                    hst_pool.tile([64, N], f32r, name="hstage", tag="hstage")
                    if j == 1
                    else None
                )
                for span in range(NSPAN):
                    po = opool.tile([65, 512], f32, tag="o")
                    ets = []
                    for half in range(8):
                        ps = spool.tile([128, 2, 512], f32, tag="s")
                        for u in range(2):
                            kc = 2 * half + u
                            nc.tensor.matmul(
                                ps[:, u, :],
                                r(kT[:, kc * 128 : (kc + 1) * 128]),
                                r(qT[:, span * 512 : (span + 1) * 512]),
                                start=True,
                                stop=True,
                            )
                        et = exp_pool.tile([128, 2, 512], f32r)
                        nc.scalar.activation(
                            et[:, :, :], ps[:, :, :], ActF.Exp, scale=float(SCALE)
                        )
                        ets.append(et)
                        if half >= 1:
                            pet = ets[half - 1]
                            for u in range(2):
                                kc = 2 * (half - 1) + u
                                nc.tensor.matmul(
                                    po[:, :],
                                    r(vaug[:, kc, j * 65 : (j + 1) * 65]),
                                    r(pet[:, u, :]),
                                    start=(kc == 0),
                                    stop=False,
                                )
                    pet = ets[7]
                    for u in range(2):
                        kc = 14 + u
                        nc.tensor.matmul(
                            po[:, :],
                            r(vaug[:, kc, j * 65 : (j + 1) * 65]),
                            r(pet[:, u, :]),
                            start=False,
                            stop=(kc == 15),
                        )
                    # normalize: copy to SBUF (DVE can read only one PSUM
                    # operand), recip of denominator row, PE-broadcast, divide
                    sb_o = sbo_pool.tile([65, 512], f32)
                    nc.vector.tensor_copy(out=sb_o[:, :], in_=po[:, :])
                    rs = rcp_pool.tile([65, 512], f32r)
                    with nc.allow_low_precision(reason="fp32r recip for matmul bcast"):
                        nc.vector.reciprocal(rs[64:65, :], sb_o[64:65, :])
                    pb = opool.tile([65, 512], f32, tag="o")
                    nc.tensor.matmul(
                        pb[0:64, :],
                        r(ones65[64:65, :]),
                        r(rs[64:65, :]),
                        start=True,
                        stop=True,
                    )
                    if j == 0:
                        ddst = attn_outT[0:64, hp, span * 512 : (span + 1) * 512]
                    else:
                        ddst = hstage[:, span * 512 : (span + 1) * 512]
                    nc.vector.tensor_tensor(
                        out=ddst,
                        in0=sb_o[0:64, :],
                        in1=pb[0:64, :],
                        op=Alu.mult,
                    )
                if j == 1:
                    nc.sync.dma_start(attn_outT[64:128, hp, :], hstage[:, :])

        # ---- output projection: outT[m*128:(m+1)*128, :] ----
        ost_pool = ctx.enter_context(tc.tile_pool(name="ostage", bufs=2))
        for m in range(DIM // 128):
            ostage = ost_pool.tile([128, N], f32)
            for sp2 in range(NSPAN // 2):
                ps = spool.tile([128, 2, 512], f32, tag="s")
                for u in range(2):
                    span = 2 * sp2 + u
                    for c in range(3):
                        nc.tensor.matmul(
                            ps[:, u, :],
                            r(wout_sb[:, c, m * 128 : (m + 1) * 128]),
                            r(attn_outT[:, c, span * 512 : (span + 1) * 512]),
                            start=(c == 0),
                            stop=(c == 2),
                        )
                nc.vector.tensor_copy(
                    out=ostage[:, sp2 * 1024 : (sp2 + 1) * 1024].rearrange(
                        "p (a b) -> p a b", a=2
                    ),
                    in_=ps[:, :, :],
                )
            nc.sync.dma_start(out_T[m * 128 : (m + 1) * 128, :], ostage[:, :])

    nc.compile()
    return nc


def _get_program():
    global _PROGRAM
    if _PROGRAM is None:
        _PROGRAM = _build_program()
    return _PROGRAM


def _round_to_f32r(a):
    """Round fp32 to the PE's fp32r format: 11-bit mantissa, low 12 bits zero
    (round to nearest, ties away handled approximately via +0x7FF + lsb)."""
    u = np.ascontiguousarray(a, dtype=np.float32).view(np.uint32)
    r = u + np.uint32(0x7FF) + ((u >> np.uint32(12)) & np.uint32(1))
    r &= np.uint32(0xFFFFF000)
    return r.view(np.float32)


def make_core_inputs(x, w_qkv, b_qkv, w_out):
    """Host-side shard: per-core input dicts for cores 0..7."""
    x = np.asarray(x, dtype=np.float32)
    w_qkv = np.asarray(w_qkv, dtype=np.float32)
    b_qkv = np.asarray(b_qkv, dtype=np.float32)
    w_out = np.asarray(w_out, dtype=np.float32)

    per_group = []
    for g in range(2):
        rows = np.concatenate(
            [
                w_qkv[qkv * DIM + g * FEAT : qkv * DIM + (g + 1) * FEAT]
                for qkv in range(3)
            ],
            axis=0,
        )  # [1152, 768]
        wqkvT_g = _round_to_f32r(rows.T)  # [768, 1152]
        b_rows = np.concatenate(
            [
                b_qkv[qkv * DIM + g * FEAT : qkv * DIM + (g + 1) * FEAT]
                for qkv in range(3)
            ],
            axis=0,
        )  # [1152]
        bias_g = np.ascontiguousarray(b_rows.reshape(9, 128).T)  # [128, 9]
        woutT_g = _round_to_f32r(w_out[:, g * FEAT : (g + 1) * FEAT].T)
        per_group.append((wqkvT_g, bias_g, woutT_g))

    in_maps = []
    for c in range(NCORES):
        b, g = c // 2, c % 2
        wqkvT_g, bias_g, woutT_g = per_group[g]
        in_maps.append(
            {
                "x": np.ascontiguousarray(x[b]),
                "wqkvT": wqkvT_g,
                "bqkv": bias_g,
                "woutT": woutT_g,
            }
        )
    return in_maps


def assemble_output(results, b_out):
    """Host-side unshard: sum partials per batch pair, transpose, add bias."""
    b_out = np.asarray(b_out, dtype=np.float32)
    out = np.empty((B, N, DIM), dtype=np.float32)
    for b in range(B):
        pT = results[2 * b]["outT"] + results[2 * b + 1]["outT"]  # [768, 2048]
        out[b] = pT.T + b_out[None, :]
    return out


def kernel(x, w_qkv, b_qkv, w_out, b_out):
    from concourse.bass_utils import run_bass_kernel_spmd

    nc = _get_program()
    in_maps = make_core_inputs(x, w_qkv, b_qkv, w_out)
    res = run_bass_kernel_spmd(nc, in_maps, list(range(NCORES)))
    return assemble_output(res.results, b_out)
